# revision 26
# baseline (speedup 1.0000x reference)
"""Fused multi-head attention block (QKV proj + per-head RMSNorm + RoPE +
softmax attention + output proj) on 8 Trainium2 NeuronCores.

Sharding: core c handles (batch b = c//2, head-group hg = c%2 of 8 heads).
Each core computes a partial output projection over its 8 heads; the host
sums the two partials per batch.

v2: q/k projections run with the weight as the stationary operand so the
outputs land directly in (d, t) layout — no PE transposes. RMS stats come
from a ones-column matmul over ACT-squared psum; rotate-half is a pair of
partition-shifted SBUF->SBUF DMAs. Softmax denominator uses a 2-level DVE
pair-sum before the ones-matmul. Output is written bf16 and converted on
the host.

Self-contained: hardcodes B=4, T=2048, C=2048, H=16, D=128.
"""

import math
import sys
import types

import numpy as np
import ml_dtypes

import concourse.bass as bass
import concourse.bacc as bacc
import concourse.tile as tile
from concourse import mybir
from concourse.bass_utils import run_bass_kernel_spmd

BF16 = mybir.dt.bfloat16
F32 = mybir.dt.float32
NP_BF16 = ml_dtypes.bfloat16
AF = mybir.ActivationFunctionType
ALU = mybir.AluOpType
AX = mybir.AxisListType

B, T, C, H, D = 4, 2048, 2048, 16, 128
HL = H // 2  # heads per core
EPS = 1e-6
NCORES = 8
HD = D // 2  # rotate-half split


def build(T_=T):
    """Build + compile the per-core Bass program (identical on all cores)."""
    nt = T_ // 128  # number of 128-row t-tiles
    ng = T_ // 512  # number of 512-col t-chunks
    ncl = C // 128  # contraction tiles over C

    nc = bacc.Bacc("TRN2", target_bir_lowering=False, debug=False, num_devices=NCORES)

    xt = nc.dram_tensor("xt", [C, T_], BF16, kind="ExternalInput")  # x[b].T
    wqk = nc.dram_tensor("wqk", [2, HL, C, D], BF16, kind="ExternalInput")  # lhsT q,k
    wv = nc.dram_tensor("wv", [C, HL * D], BF16, kind="ExternalInput")  # moving v
    wp = nc.dram_tensor("wp", [HL, D, C], BF16, kind="ExternalInput")  # (h, dv, o)
    cqT = nc.dram_tensor("cqT", [D, T_], BF16, kind="ExternalInput")
    sqT = nc.dram_tensor("sqT", [D, T_], BF16, kind="ExternalInput")
    ckT = nc.dram_tensor("ckT", [D, T_], BF16, kind="ExternalInput")
    skT = nc.dram_tensor("skT", [D, T_], BF16, kind="ExternalInput")
    out = nc.dram_tensor("out", [T_, C], BF16, kind="ExternalOutput")

    with tile.TileContext(nc) as tc:
        with (
            tc.tile_pool(name="persist", bufs=1) as persist,
            tc.tile_pool(name="dram", bufs=1, space="DRAM") as dpool,
        ):
            ones_b = persist.tile([128, 1], BF16)
            nc.vector.memset(ones_b[:], 1.0)
            eps_b = persist.tile([128, 1], F32)
            nc.vector.memset(eps_b[:], EPS)

            KT = persist.tile([128, HL, T_], BF16)  # k^T: (d, h, t)
            Vs = persist.tile([128, HL, nt, D], BF16)  # v: (t-part, h, t-tile, dv)
            qt_dram = dpool.tile([HL, D, T_], BF16)  # q^T spill: (h, d, t)

            # ---------- phase 1: QKV proj + RMS norm + RoPE ----------
            with (
                tc.tile_pool(name="w_pool", bufs=2) as w_pool,
                tc.tile_pool(name="x_pool", bufs=2) as x_pool,
                tc.tile_pool(name="cs_pool", bufs=1) as cs_pool,
                tc.tile_pool(name="work", bufs=2) as work,
                tc.tile_pool(name="qk_ps", bufs=5, space="PSUM") as qk_ps,
                tc.tile_pool(name="ss_ps", bufs=3, space="PSUM") as ss_ps,
            ):
                # rope tables, transposed to (d, t), loaded once (lazily, so
                # the first x panel isn't stuck behind them on the ACT queue)
                cst = {}

                def get_cs(nm, dr):
                    if nm not in cst:
                        t_ = cs_pool.tile([128, T_], BF16, tag=nm, name=nm)
                        nc.scalar.dma_start(out=t_[:], in_=dr[:])
                        cst[nm] = t_
                    return cst[nm]

                def load_xp(ch):
                    """x panel for one 512-col t-chunk, via the ACT queue."""
                    xp = x_pool.tile([128, ncl, 512], BF16, tag="xp")
                    for cc in range(4):
                        nc.scalar.dma_start(
                            out=xp[:, cc * 4 : (cc + 1) * 4, :],
                            in_=xt[:]
                            .rearrange("(n p) t -> p n t", p=128)[
                                :, cc * 4 : (cc + 1) * 4, ch * 512 : (ch + 1) * 512
                            ],
                        )
                    return xp

                def emit_rope(fg, h, ch, ps, cos_t, sin_t):
                    """RMS norm + RoPE for one finished q/k psum tile (d, t)."""
                    sl = slice(ch * 512, (ch + 1) * 512)
                    # sum of squares over d (partitions) via ones-matmul
                    sq = work.tile([128, 512], BF16, tag="sq")
                    nc.scalar.activation(sq[:], ps[:], AF.Square)
                    ss = ss_ps.tile([1, 512], F32, tag="ss")
                    nc.tensor.matmul(
                        ss[:], lhsT=ones_b[:], rhs=sq[:], start=True, stop=True
                    )
                    srt = work.tile([1, 512], F32, tag="srt")
                    nc.scalar.activation(
                        srt[:], ss[:], AF.Sqrt, scale=1.0 / D, bias=eps_b[0:1, :]
                    )
                    rstd = work.tile([1, 512], F32, tag="rstd")
                    nc.vector.reciprocal_approx_fast(rstd[:], srt[:])
                    rstd_b = work.tile([128, 512], F32, tag="rstdb")
                    nc.gpsimd.partition_broadcast(rstd_b[:], rstd[:])
                    # normalize (reads psum fp32, writes bf16)
                    qn = work.tile([128, 512], BF16, tag="qn")
                    nc.vector.tensor_mul(qn[:], ps[:], rstd_b[:])
                    # rotate-half via partition-shifted SBUF->SBUF DMAs
                    rot = work.tile([128, 512], BF16, tag="rot")
                    nc.sync.dma_start(out=rot[0:HD, :], in_=qn[HD:D, :])
                    nc.sync.dma_start(out=rot[HD:D, :], in_=qn[0:HD, :])
                    # rope: out = qn*cos + rot*sin  (sign folded into sin)
                    m1 = work.tile([128, 512], BF16, tag="m1")
                    nc.vector.tensor_mul(m1[:], qn[:], cos_t[:, sl])
                    m2 = work.tile([128, 512], BF16, tag="m2")
                    nc.vector.tensor_mul(m2[:], rot[:], sin_t[:, sl])
                    if fg == 1:
                        nc.vector.tensor_add(KT[:, h, sl], m1[:], m2[:])
                    else:
                        qs = work.tile([128, 512], BF16, tag="qs")
                        nc.vector.tensor_add(qs[:], m1[:], m2[:])
                        nc.sync.dma_start(out=qt_dram[h, :, sl], in_=qs[:])

                pend = []
                fgs = (1, 2, 0)  # k, v, q (q last so attention follows)
                # prefetch chain over all (fg, ch) x panels: panel i+1 loads
                # while panel i's first task computes, keeping the panel DMA
                # ahead of the ACT queue's per-task compute ops
                n_panels = len(fgs) * ng
                xp_next = load_xp(0)
                for fgi, fg in enumerate(fgs):
                    if fg != 2:
                        wts = []
                        for h in range(HL):
                            wt = w_pool.tile([128, ncl, D], BF16, tag="wt", bufs=9)
                            src = wqk[fg, h].rearrange("(n p) d -> p n d", p=128)
                            if fg == 1 and h == 0:
                                # chunk the very first load so the first
                                # matmuls aren't gated on the full 512 KB
                                for cc in range(4):
                                    nc.sync.dma_start(
                                        out=wt[:, cc * 4 : (cc + 1) * 4, :],
                                        in_=src[:, cc * 4 : (cc + 1) * 4, :],
                                    )
                            else:
                                nc.sync.dma_start(out=wt[:], in_=src)
                            wts.append(wt)
                        cos_t = sin_t = None
                        for ch in range(ng):
                            xp = xp_next
                            if cos_t is None:
                                cos_t = get_cs(*(("cq", cqT) if fg == 0 else ("ck", ckT)))
                                sin_t = get_cs(*(("sq", sqT) if fg == 0 else ("sk", skT)))
                            for h in range(HL):
                                ps = qk_ps.tile([128, 512], F32, tag="qk")
                                for ct in range(ncl):
                                    nc.tensor.matmul(
                                        ps[:],
                                        lhsT=wts[h][:, ct, :],
                                        rhs=xp[:, ct, :],
                                        start=(ct == 0),
                                        stop=(ct == ncl - 1),
                                    )
                                if h == 0 and fgi * ng + ch + 1 < n_panels:
                                    xp_next = load_xp((ch + 1) % ng)
                                pend.append((fg, h, ch, ps, cos_t, sin_t))
                                if len(pend) >= 2:
                                    emit_rope(*pend.pop(0))
                        # flush before the next fg pass: later-emitted engine
                        # work must not depend on un-emitted rope ops
                        while pend:
                            emit_rope(*pend.pop(0))
                    else:
                        wvs = []
                        for half in range(2):
                            wv_t = w_pool.tile([128, ncl, 512], BF16, tag="wv")
                            nc.sync.dma_start(
                                out=wv_t[:],
                                in_=wv[:].rearrange("(n p) f -> p n f", p=128)[
                                    :, :, half * 512 : (half + 1) * 512
                                ],
                            )
                            wvs.append(wv_t)
                        for ch in range(ng):
                            xp = xp_next
                            for ts in range(4):
                                for half in range(2):
                                    ps = qk_ps.tile([128, 512], F32, tag="qk")
                                    for ct in range(ncl):
                                        nc.tensor.matmul(
                                            ps[:],
                                            lhsT=xp[:, ct, ts * 128 : (ts + 1) * 128],
                                            rhs=wvs[half][:, ct, :],
                                            start=(ct == 0),
                                            stop=(ct == ncl - 1),
                                        )
                                    if ts == 0 and half == 1:
                                        if fgi * ng + ch + 1 < n_panels:
                                            xp_next = load_xp((ch + 1) % ng)
                                    nc.scalar.copy(
                                        Vs[
                                            :,
                                            half * 4 : (half + 1) * 4,
                                            ch * 4 + ts,
                                            :,
                                        ],
                                        ps[:].rearrange("p (h d) -> p h d", h=4),
                                    )

            # ---------- phase 2: attention + output projection ----------
            with (
                tc.tile_pool(name="wp_pool", bufs=1) as wp_pool,
                tc.tile_pool(name="qt_pool", bufs=2) as qt_pool,
                tc.tile_pool(name="pt_pool", bufs=3) as pt_pool,
                tc.tile_pool(name="y_pool", bufs=2) as y_pool,
                tc.tile_pool(name="pa_pool", bufs=2) as pa_pool,
                tc.tile_pool(name="o_pool", bufs=1) as o_pool,
                tc.tile_pool(name="r_pool", bufs=1) as r_pool,
                tc.tile_pool(name="sp_ps", bufs=2, space="PSUM") as sp_ps,
                tc.tile_pool(name="acc_ps", bufs=4, space="PSUM") as acc_ps,
            ):
                def load_qtc(ch):
                    qtc = qt_pool.tile([128, HL, 512], BF16, tag="qtc")
                    nc.sync.dma_start(
                        out=qtc[:],
                        in_=qt_dram[:, :, ch * 512 : (ch + 1) * 512].rearrange(
                            "h d t -> d h t"
                        ),
                    )
                    return qtc

                # first-chunk q tiles before the (large) proj-weight load so
                # the first scores matmuls aren't stuck behind it; WPT rides
                # the ACT queue so its 4.2 MB doesn't block later qtc loads
                qtc_next = load_qtc(0)
                WPT = wp_pool.tile([128, HL, C], BF16)
                nc.scalar.dma_start(out=WPT[:], in_=wp[:].rearrange("h d o -> d h o"))

                from collections import deque

                pe_work = deque()  # closures emitting ~2-4 PE MMs each,
                # drained between score pairs so exp-gated score stretches
                # never leave the PE idle while PV/denom/proj work exists

                def drain(k):
                    for _ in range(k):
                        if not pe_work:
                            return
                        pe_work.popleft()()

                def emit_scores(qtc, h, PT):
                    """S^T = K^T.T @ q^T for one head/chunk; exp into PT."""
                    for sg in range(nt // 2):
                        sp = sp_ps.tile([128, 2, 512], F32, tag="sp")
                        for i in range(2):
                            tk = sg * 2 + i
                            nc.tensor.matmul(
                                sp[:, i, :],
                                lhsT=KT[:, h, tk * 128 : (tk + 1) * 128],
                                rhs=qtc[:, h, :],
                                start=True,
                                stop=True,
                            )
                        nc.scalar.activation(
                            PT[:, sg * 2 : sg * 2 + 2, :], sp[:], AF.Exp
                        )
                        drain(2)

                def emit_adds(t):
                    """3-level DVE pair-sums of a finished task's PT tiles
                    (eighths the denominator matmul stream)."""
                    pa = pa_pool.tile([128, nt // 2, 512], BF16, tag="pa")
                    t["pa"] = pa
                    for sg in range(nt // 2):
                        nc.vector.tensor_add(
                            pa[:, sg, :],
                            t["PT"][:, sg * 2, :],
                            t["PT"][:, sg * 2 + 1, :],
                        )
                    for sg in range(nt // 4):
                        nc.vector.tensor_add(
                            pa[:, sg, :], pa[:, sg * 2, :], pa[:, sg * 2 + 1, :]
                        )
                    for sg in range(nt // 8):
                        nc.vector.tensor_add(
                            pa[:, sg, :], pa[:, sg * 2, :], pa[:, sg * 2 + 1, :]
                        )

                def gen_consume(t):
                    """Queue PV + denominator + normalize for one head/chunk
                    as fine-grained PE work items."""
                    h, PT, Ysb = t["h"], t["PT"], t["Ysb"]
                    box = {}

                    def pv(i0):
                        def f():
                            if i0 == 0:
                                box["yp"] = acc_ps.tile([128, 512], F32, tag="acc", name="yp")
                            for i in (i0, i0 + 1):
                                nc.tensor.matmul(
                                    box["yp"][:],
                                    lhsT=Vs[:, h, i, :],
                                    rhs=PT[:, i, :],
                                    start=(i == 0),
                                    stop=(i == nt - 1),
                                )
                        return f

                    for i0 in range(0, nt, 2):
                        pe_work.append(pv(i0))

                    def fin():
                        pa = t["pa"]
                        ss = acc_ps.tile([1, 512], F32, tag="acc")
                        for i in range(nt // 8):
                            nc.tensor.matmul(
                                ss[:],
                                lhsT=ones_b[:],
                                rhs=pa[:, i, :],
                                start=(i == 0),
                                stop=(i == nt // 8 - 1),
                            )
                        rinv = r_pool.tile([1, 512], F32, tag="rinv")
                        nc.vector.reciprocal_approx_fast(rinv[:], ss[:])
                        rbs = r_pool.tile([128, 512], F32, tag="rbs")
                        nc.gpsimd.partition_broadcast(rbs[:], rinv[:])
                        nc.vector.tensor_mul(Ysb[:, h, :], box["yp"][:], rbs[:])

                    pe_work.append(fin)
                    if h == HL - 1:
                        gen_proj(t["ch"], Ysb)

                def gen_proj(ch, Ysb):
                    """Queue the partial output projection for one chunk."""
                    for ts in range(4):
                        box = {}

                        def pj(ts, ot, half):
                            def f():
                                if ot == 0 and half == 0:
                                    box["osb"] = o_pool.tile([128, C], BF16, tag="osb", name="osb")
                                if half == 0:
                                    box["op"] = acc_ps.tile([128, 512], F32, tag="acc", name="op")
                                for h in range(half * 4, half * 4 + 4):
                                    nc.tensor.matmul(
                                        box["op"][:],
                                        lhsT=Ysb[:, h, ts * 128 : (ts + 1) * 128],
                                        rhs=WPT[:, h, ot * 512 : (ot + 1) * 512],
                                        start=(h == 0),
                                        stop=(h == HL - 1),
                                    )
                                if half == 1:
                                    nc.vector.tensor_copy(
                                        box["osb"][:, ot * 512 : (ot + 1) * 512],
                                        box["op"][:],
                                    )
                                    if ot == C // 512 - 1:
                                        nc.sync.dma_start(
                                            out=out[:].rearrange(
                                                "(n p) o -> n p o", p=128
                                            )[ch * 4 + ts],
                                            in_=box["osb"][:],
                                        )
                            return f

                        for ot in range(C // 512):
                            for half in range(2):
                                pe_work.append(pj(ts, ot, half))

                pending = []  # task dicts, consumed three iterations later

                for ch in range(ng):
                    qtc = qtc_next
                    Ysb = y_pool.tile([128, HL, 512], BF16, tag="y")
                    for h in range(HL):
                        PT = pt_pool.tile([128, nt, 512], BF16, tag="pt")
                        emit_scores(qtc, h, PT)
                        if h == 0 and ch + 1 < ng:
                            qtc_next = load_qtc(ch + 1)
                        if pending:
                            emit_adds(pending[-1])
                        if len(pending) == 3:
                            gen_consume(pending.pop(0))
                        pending.append(
                            {"ch": ch, "h": h, "PT": PT, "pa": None, "Ysb": Ysb}
                        )
                emit_adds(pending[-1])
                while pending:
                    gen_consume(pending.pop(0))
                drain(len(pe_work))

    nc.compile()
    return nc


def prep_inputs(x, cos, sin, w_qkv, w_proj, g_q, g_k, T_=T, b_count=B):
    """Host-side sharding: per-core input dicts."""
    x = np.asarray(x, dtype=np.float32)
    cos = np.asarray(cos, dtype=np.float32)[:T_]
    sin = np.asarray(sin, dtype=np.float32)[:T_]
    w_qkv = np.asarray(w_qkv, dtype=np.float32)
    w_proj = np.asarray(w_proj, dtype=np.float32)
    g_q = np.asarray(g_q, dtype=np.float32)
    g_k = np.asarray(g_k, dtype=np.float32)

    srcidx = np.concatenate([np.arange(HD, D), np.arange(0, HD)])
    sign = np.concatenate([-np.ones(HD, np.float32), np.ones(HD, np.float32)])
    scale_q = 1.0 / math.sqrt(D)
    # transposed (d, t) rope tables; rotation sign folded into sin
    cqT_np = np.ascontiguousarray((cos * g_q[None, :] * scale_q).T).astype(NP_BF16)
    sqT_np = np.ascontiguousarray(
        (sin * sign[None, :] * g_q[srcidx][None, :] * scale_q).T
    ).astype(NP_BF16)
    ckT_np = np.ascontiguousarray((cos * g_k[None, :]).T).astype(NP_BF16)
    skT_np = np.ascontiguousarray((sin * sign[None, :] * g_k[srcidx][None, :]).T).astype(
        NP_BF16
    )

    wq_r = w_qkv.reshape(3, H, D, C)
    wp_r = w_proj.reshape(C, H, D)

    in_maps = []
    for core in range(NCORES):
        b = core // 2
        hg = core % 2
        xt_np = np.ascontiguousarray(x[b % b_count][:T_].T).astype(NP_BF16)
        wsel = wq_r[:, hg * HL : (hg + 1) * HL]  # [3, HL, D, C]
        # q,k as stationary lhsT tiles: [2, HL, C, D]
        wqk_np = np.ascontiguousarray(wsel[0:2].transpose(0, 1, 3, 2)).astype(NP_BF16)
        # v as moving rhs: [C, HL*D]
        wv_np = np.ascontiguousarray(
            wsel[2].transpose(2, 0, 1).reshape(C, HL * D)
        ).astype(NP_BF16)
        wp_np = np.ascontiguousarray(
            wp_r[:, hg * HL : (hg + 1) * HL, :].transpose(1, 2, 0)
        ).astype(NP_BF16)  # [HL, D, C]
        in_maps.append(
            {
                "xt": xt_np,
                "wqk": wqk_np,
                "wv": wv_np,
                "wp": wp_np,
                "cqT": cqT_np,
                "sqT": sqT_np,
                "ckT": ckT_np,
                "skT": skT_np,
            }
        )
    return in_maps


_nc_cache = {}


def _get_nc(T_=T):
    if T_ not in _nc_cache:
        _nc_cache[T_] = build(T_)
    return _nc_cache[T_]


def _install_trace_hook():
    """Register the axon NTFF profile hook (missing from this image's antenv)."""
    if "antenv.axon_hooks" in sys.modules:
        return
    try:
        from trn_agent_boot.trn_boot import _ntff_profile_via_ctypes

        hook = _ntff_profile_via_ctypes("/opt/axon/libaxon_pjrt.so")
        mod = types.ModuleType("antenv.axon_hooks")
        mod.get_axon_ntff_profile_hook = lambda: hook
        sys.modules["antenv.axon_hooks"] = mod
    except Exception:
        pass


def run(inputs, T_=T, trace=False, tmpdir=None):
    """Run the sharded kernel; returns (full output [B, T, C] fp32, results obj)."""
    nc = _get_nc(T_)
    in_maps = prep_inputs(**inputs, T_=T_)
    kwargs = {}
    if trace:
        _install_trace_hook()
        kwargs = dict(trace=True, tmpdir=tmpdir)
    res = run_bass_kernel_spmd(nc, in_maps, core_ids=list(range(NCORES)), **kwargs)
    outs = [np.asarray(res.results[i]["out"], dtype=np.float32) for i in range(NCORES)]
    full = np.stack([outs[2 * b] + outs[2 * b + 1] for b in range(B)], axis=0)
    return full, res


def kernel(x, cos, sin, w_qkv, w_proj, g_q, g_k):
    full, _ = run(
        dict(x=x, cos=cos, sin=sin, w_qkv=w_qkv, w_proj=w_proj, g_q=g_q, g_k=g_k)
    )
    return full


# revision 27
# speedup vs baseline: 1.0144x; 1.0144x over previous
"""Fused multi-head attention block (QKV proj + per-head RMSNorm + RoPE +
softmax attention + output proj) on 8 Trainium2 NeuronCores.

Sharding: core c handles (batch b = c//2, head-group hg = c%2 of 8 heads).
Each core computes a partial output projection over its 8 heads; the host
sums the two partials per batch.

v2: q/k projections run with the weight as the stationary operand so the
outputs land directly in (d, t) layout — no PE transposes. RMS stats come
from a ones-column matmul over ACT-squared psum; rotate-half is a pair of
partition-shifted SBUF->SBUF DMAs. Softmax denominator uses a 2-level DVE
pair-sum before the ones-matmul. Output is written bf16 and converted on
the host.

Self-contained: hardcodes B=4, T=2048, C=2048, H=16, D=128.
"""

import math
import sys
import types

import numpy as np
import ml_dtypes

import concourse.bass as bass
import concourse.bacc as bacc
import concourse.tile as tile
from concourse import mybir
from concourse.bass_utils import run_bass_kernel_spmd

BF16 = mybir.dt.bfloat16
F32 = mybir.dt.float32
NP_BF16 = ml_dtypes.bfloat16
AF = mybir.ActivationFunctionType
ALU = mybir.AluOpType
AX = mybir.AxisListType

B, T, C, H, D = 4, 2048, 2048, 16, 128
HL = H // 2  # heads per core
EPS = 1e-6
NCORES = 8
HD = D // 2  # rotate-half split


def build(T_=T):
    """Build + compile the per-core Bass program (identical on all cores)."""
    nt = T_ // 128  # number of 128-row t-tiles
    ng = T_ // 512  # number of 512-col t-chunks
    ncl = C // 128  # contraction tiles over C

    nc = bacc.Bacc("TRN2", target_bir_lowering=False, debug=False, num_devices=NCORES)

    xt = nc.dram_tensor("xt", [C, T_], BF16, kind="ExternalInput")  # x[b].T
    wqk = nc.dram_tensor("wqk", [2, HL, C, D], BF16, kind="ExternalInput")  # lhsT q,k
    wv = nc.dram_tensor("wv", [C, HL * D], BF16, kind="ExternalInput")  # moving v
    wp = nc.dram_tensor("wp", [HL, D, C], BF16, kind="ExternalInput")  # (h, dv, o)
    cqT = nc.dram_tensor("cqT", [D, T_], BF16, kind="ExternalInput")
    sqT = nc.dram_tensor("sqT", [D, T_], BF16, kind="ExternalInput")
    ckT = nc.dram_tensor("ckT", [D, T_], BF16, kind="ExternalInput")
    skT = nc.dram_tensor("skT", [D, T_], BF16, kind="ExternalInput")
    out = nc.dram_tensor("out", [T_, C], BF16, kind="ExternalOutput")

    with tile.TileContext(nc) as tc:
        with (
            tc.tile_pool(name="persist", bufs=1) as persist,
            tc.tile_pool(name="dram", bufs=1, space="DRAM") as dpool,
        ):
            ones_b = persist.tile([128, 1], BF16)
            nc.vector.memset(ones_b[:], 1.0)
            eps_b = persist.tile([128, 1], F32)
            nc.vector.memset(eps_b[:], EPS)

            KT = persist.tile([128, HL, T_], BF16)  # k^T: (d, h, t)
            Vs = persist.tile([128, HL, nt, D], BF16)  # v: (t-part, h, t-tile, dv)
            qt_dram = dpool.tile([HL, D, T_], BF16)  # q^T spill: (h, d, t)

            # ---------- phase 1: QKV proj + RMS norm + RoPE ----------
            with (
                tc.tile_pool(name="w_pool", bufs=2) as w_pool,
                tc.tile_pool(name="x_pool", bufs=2) as x_pool,
                tc.tile_pool(name="cs_pool", bufs=1) as cs_pool,
                tc.tile_pool(name="work", bufs=2) as work,
                tc.tile_pool(name="qk_ps", bufs=5, space="PSUM") as qk_ps,
                tc.tile_pool(name="ss_ps", bufs=3, space="PSUM") as ss_ps,
            ):
                # rope tables, transposed to (d, t), loaded once (lazily, so
                # the first x panel isn't stuck behind them on the ACT queue)
                cst = {}

                def get_cs(nm, dr):
                    if nm not in cst:
                        t_ = cs_pool.tile([128, T_], BF16, tag=nm, name=nm)
                        nc.scalar.dma_start(out=t_[:], in_=dr[:])
                        cst[nm] = t_
                    return cst[nm]

                def load_xp(ch):
                    """x panel for one 512-col t-chunk, via the ACT queue."""
                    xp = x_pool.tile([128, ncl, 512], BF16, tag="xp")
                    for cc in range(4):
                        nc.scalar.dma_start(
                            out=xp[:, cc * 4 : (cc + 1) * 4, :],
                            in_=xt[:]
                            .rearrange("(n p) t -> p n t", p=128)[
                                :, cc * 4 : (cc + 1) * 4, ch * 512 : (ch + 1) * 512
                            ],
                        )
                    return xp

                def emit_rope(fg, h, ch, ps, cos_t, sin_t):
                    """RMS norm + RoPE for one finished q/k psum tile (d, t)."""
                    sl = slice(ch * 512, (ch + 1) * 512)
                    # sum of squares over d (partitions) via ones-matmul
                    sq = work.tile([128, 512], BF16, tag="sq")
                    nc.scalar.activation(sq[:], ps[:], AF.Square)
                    ss = ss_ps.tile([1, 512], F32, tag="ss")
                    nc.tensor.matmul(
                        ss[:], lhsT=ones_b[:], rhs=sq[:], start=True, stop=True
                    )
                    srt = work.tile([1, 512], F32, tag="srt")
                    nc.scalar.activation(
                        srt[:], ss[:], AF.Sqrt, scale=1.0 / D, bias=eps_b[0:1, :]
                    )
                    rstd = work.tile([1, 512], F32, tag="rstd")
                    nc.vector.reciprocal_approx_fast(rstd[:], srt[:])
                    rstd_b = work.tile([128, 512], F32, tag="rstdb")
                    nc.gpsimd.partition_broadcast(rstd_b[:], rstd[:])
                    # normalize (reads psum fp32, writes bf16)
                    qn = work.tile([128, 512], BF16, tag="qn")
                    nc.vector.tensor_mul(qn[:], ps[:], rstd_b[:])
                    # rotate-half via partition-shifted SBUF->SBUF DMAs
                    rot = work.tile([128, 512], BF16, tag="rot")
                    nc.sync.dma_start(out=rot[0:HD, :], in_=qn[HD:D, :])
                    nc.sync.dma_start(out=rot[HD:D, :], in_=qn[0:HD, :])
                    # rope: out = qn*cos + rot*sin  (sign folded into sin)
                    m1 = work.tile([128, 512], BF16, tag="m1")
                    nc.vector.tensor_mul(m1[:], qn[:], cos_t[:, sl])
                    m2 = work.tile([128, 512], BF16, tag="m2")
                    nc.vector.tensor_mul(m2[:], rot[:], sin_t[:, sl])
                    if fg == 1:
                        nc.vector.tensor_add(KT[:, h, sl], m1[:], m2[:])
                    else:
                        qs = work.tile([128, 512], BF16, tag="qs")
                        nc.vector.tensor_add(qs[:], m1[:], m2[:])
                        nc.sync.dma_start(out=qt_dram[h, :, sl], in_=qs[:])

                pend = []
                fgs = (1, 2, 0)  # k, v, q (q last so attention follows)
                # prefetch chain over all (fg, ch) x panels: panel i+1 loads
                # while panel i's first task computes, keeping the panel DMA
                # ahead of the ACT queue's per-task compute ops
                n_panels = len(fgs) * ng
                xp_next = load_xp(0)
                for fgi, fg in enumerate(fgs):
                    if fg != 2:
                        wts = []
                        for h in range(HL):
                            wt = w_pool.tile([128, ncl, D], BF16, tag="wt", bufs=9)
                            src = wqk[fg, h].rearrange("(n p) d -> p n d", p=128)
                            if fg == 1 and h == 0:
                                # chunk the very first load so the first
                                # matmuls aren't gated on the full 512 KB
                                for cc in range(4):
                                    nc.sync.dma_start(
                                        out=wt[:, cc * 4 : (cc + 1) * 4, :],
                                        in_=src[:, cc * 4 : (cc + 1) * 4, :],
                                    )
                            else:
                                nc.sync.dma_start(out=wt[:], in_=src)
                            wts.append(wt)
                        cos_t = sin_t = None
                        for ch in range(ng):
                            xp = xp_next
                            if cos_t is None:
                                cos_t = get_cs(*(("cq", cqT) if fg == 0 else ("ck", ckT)))
                                sin_t = get_cs(*(("sq", sqT) if fg == 0 else ("sk", skT)))
                            for h in range(HL):
                                ps = qk_ps.tile([128, 512], F32, tag="qk")
                                for ct in range(ncl):
                                    nc.tensor.matmul(
                                        ps[:],
                                        lhsT=wts[h][:, ct, :],
                                        rhs=xp[:, ct, :],
                                        start=(ct == 0),
                                        stop=(ct == ncl - 1),
                                    )
                                if h == 0 and fgi * ng + ch + 1 < n_panels:
                                    xp_next = load_xp((ch + 1) % ng)
                                pend.append((fg, h, ch, ps, cos_t, sin_t))
                                if len(pend) >= 2:
                                    emit_rope(*pend.pop(0))
                        # flush before the next fg pass: later-emitted engine
                        # work must not depend on un-emitted rope ops
                        while pend:
                            emit_rope(*pend.pop(0))
                    else:
                        wvs = []
                        for half in range(2):
                            wv_t = w_pool.tile([128, ncl, 512], BF16, tag="wv")
                            nc.sync.dma_start(
                                out=wv_t[:],
                                in_=wv[:].rearrange("(n p) f -> p n f", p=128)[
                                    :, :, half * 512 : (half + 1) * 512
                                ],
                            )
                            wvs.append(wv_t)
                        for ch in range(ng):
                            xp = xp_next
                            for ts in range(4):
                                for half in range(2):
                                    ps = qk_ps.tile([128, 512], F32, tag="qk")
                                    for ct in range(ncl):
                                        nc.tensor.matmul(
                                            ps[:],
                                            lhsT=xp[:, ct, ts * 128 : (ts + 1) * 128],
                                            rhs=wvs[half][:, ct, :],
                                            start=(ct == 0),
                                            stop=(ct == ncl - 1),
                                        )
                                    if ts == 0 and half == 1:
                                        if fgi * ng + ch + 1 < n_panels:
                                            xp_next = load_xp((ch + 1) % ng)
                                    nc.scalar.copy(
                                        Vs[
                                            :,
                                            half * 4 : (half + 1) * 4,
                                            ch * 4 + ts,
                                            :,
                                        ],
                                        ps[:].rearrange("p (h d) -> p h d", h=4),
                                    )

            # ---------- phase 2: attention + output projection ----------
            with (
                tc.tile_pool(name="wp_pool", bufs=1) as wp_pool,
                tc.tile_pool(name="qt_pool", bufs=2) as qt_pool,
                tc.tile_pool(name="pt_pool", bufs=3) as pt_pool,
                tc.tile_pool(name="y_pool", bufs=2) as y_pool,
                tc.tile_pool(name="pa_pool", bufs=2) as pa_pool,
                tc.tile_pool(name="o_pool", bufs=1) as o_pool,
                tc.tile_pool(name="r_pool", bufs=1) as r_pool,
                tc.tile_pool(name="sp_ps", bufs=2, space="PSUM") as sp_ps,
                tc.tile_pool(name="acc_ps", bufs=4, space="PSUM") as acc_ps,
            ):
                def load_qtc(ch):
                    qtc = qt_pool.tile([128, HL, 512], BF16, tag="qtc")
                    nc.sync.dma_start(
                        out=qtc[:],
                        in_=qt_dram[:, :, ch * 512 : (ch + 1) * 512].rearrange(
                            "h d t -> d h t"
                        ),
                    )
                    return qtc

                # first-chunk q tiles before the (large) proj-weight load so
                # the first scores matmuls aren't stuck behind it; WPT rides
                # the ACT queue so its 4.2 MB doesn't block later qtc loads
                qtc_next = load_qtc(0)
                WPT = wp_pool.tile([128, HL, C], BF16)
                nc.sync.dma_start(out=WPT[:], in_=wp[:].rearrange("h d o -> d h o"))

                from collections import deque

                pe_work = deque()  # closures emitting ~2-4 PE MMs each,
                # drained between score pairs so exp-gated score stretches
                # never leave the PE idle while PV/denom/proj work exists

                def drain(k):
                    for _ in range(k):
                        if not pe_work:
                            return
                        pe_work.popleft()()

                def emit_scores(qtc, h, PT):
                    """S^T = K^T.T @ q^T for one head/chunk; exp into PT."""
                    for sg in range(nt // 2):
                        sp = sp_ps.tile([128, 2, 512], F32, tag="sp")
                        for i in range(2):
                            tk = sg * 2 + i
                            nc.tensor.matmul(
                                sp[:, i, :],
                                lhsT=KT[:, h, tk * 128 : (tk + 1) * 128],
                                rhs=qtc[:, h, :],
                                start=True,
                                stop=True,
                            )
                        nc.scalar.activation(
                            PT[:, sg * 2 : sg * 2 + 2, :], sp[:], AF.Exp
                        )
                        drain(2)

                def emit_adds(t):
                    """3-level DVE pair-sums of a finished task's PT tiles
                    (eighths the denominator matmul stream)."""
                    pa = pa_pool.tile([128, nt // 2, 512], BF16, tag="pa")
                    t["pa"] = pa
                    for sg in range(nt // 2):
                        nc.vector.tensor_add(
                            pa[:, sg, :],
                            t["PT"][:, sg * 2, :],
                            t["PT"][:, sg * 2 + 1, :],
                        )
                    for sg in range(nt // 4):
                        nc.vector.tensor_add(
                            pa[:, sg, :], pa[:, sg * 2, :], pa[:, sg * 2 + 1, :]
                        )
                    for sg in range(nt // 8):
                        nc.vector.tensor_add(
                            pa[:, sg, :], pa[:, sg * 2, :], pa[:, sg * 2 + 1, :]
                        )

                def gen_consume(t):
                    """Queue PV + denominator + normalize for one head/chunk
                    as fine-grained PE work items."""
                    h, PT, Ysb = t["h"], t["PT"], t["Ysb"]
                    box = {}

                    def pv(i0):
                        def f():
                            if i0 == 0:
                                box["yp"] = acc_ps.tile([128, 512], F32, tag="acc", name="yp")
                            for i in (i0, i0 + 1):
                                nc.tensor.matmul(
                                    box["yp"][:],
                                    lhsT=Vs[:, h, i, :],
                                    rhs=PT[:, i, :],
                                    start=(i == 0),
                                    stop=(i == nt - 1),
                                )
                        return f

                    for i0 in range(0, nt, 2):
                        pe_work.append(pv(i0))

                    def fin():
                        pa = t["pa"]
                        ss = acc_ps.tile([1, 512], F32, tag="acc")
                        for i in range(nt // 8):
                            nc.tensor.matmul(
                                ss[:],
                                lhsT=ones_b[:],
                                rhs=pa[:, i, :],
                                start=(i == 0),
                                stop=(i == nt // 8 - 1),
                            )
                        rinv = r_pool.tile([1, 512], F32, tag="rinv")
                        nc.vector.reciprocal_approx_fast(rinv[:], ss[:])
                        rbs = r_pool.tile([128, 512], F32, tag="rbs")
                        nc.gpsimd.partition_broadcast(rbs[:], rinv[:])
                        nc.vector.tensor_mul(Ysb[:, h, :], box["yp"][:], rbs[:])

                    pe_work.append(fin)
                    if h == HL - 1:
                        gen_proj(t["ch"], Ysb)

                def gen_proj(ch, Ysb):
                    """Queue the partial output projection for one chunk."""
                    for ts in range(4):
                        box = {}

                        def pj(ts, ot, half):
                            def f():
                                if ot == 0 and half == 0:
                                    box["osb"] = o_pool.tile([128, C], BF16, tag="osb", name="osb")
                                if half == 0:
                                    box["op"] = acc_ps.tile([128, 512], F32, tag="acc", name="op")
                                for h in range(half * 4, half * 4 + 4):
                                    nc.tensor.matmul(
                                        box["op"][:],
                                        lhsT=Ysb[:, h, ts * 128 : (ts + 1) * 128],
                                        rhs=WPT[:, h, ot * 512 : (ot + 1) * 512],
                                        start=(h == 0),
                                        stop=(h == HL - 1),
                                    )
                                if half == 1:
                                    nc.vector.tensor_copy(
                                        box["osb"][:, ot * 512 : (ot + 1) * 512],
                                        box["op"][:],
                                    )
                                    if ot == C // 512 - 1:
                                        nc.sync.dma_start(
                                            out=out[:].rearrange(
                                                "(n p) o -> n p o", p=128
                                            )[ch * 4 + ts],
                                            in_=box["osb"][:],
                                        )
                            return f

                        for ot in range(C // 512):
                            for half in range(2):
                                pe_work.append(pj(ts, ot, half))

                pending = []  # task dicts, consumed three iterations later

                for ch in range(ng):
                    qtc = qtc_next
                    Ysb = y_pool.tile([128, HL, 512], BF16, tag="y")
                    for h in range(HL):
                        PT = pt_pool.tile([128, nt, 512], BF16, tag="pt")
                        emit_scores(qtc, h, PT)
                        if h == 0 and ch + 1 < ng:
                            qtc_next = load_qtc(ch + 1)
                        if pending:
                            emit_adds(pending[-1])
                        if len(pending) == 3:
                            gen_consume(pending.pop(0))
                        pending.append(
                            {"ch": ch, "h": h, "PT": PT, "pa": None, "Ysb": Ysb}
                        )
                emit_adds(pending[-1])
                while pending:
                    gen_consume(pending.pop(0))
                drain(len(pe_work))

    nc.compile()
    return nc


def prep_inputs(x, cos, sin, w_qkv, w_proj, g_q, g_k, T_=T, b_count=B):
    """Host-side sharding: per-core input dicts."""
    x = np.asarray(x, dtype=np.float32)
    cos = np.asarray(cos, dtype=np.float32)[:T_]
    sin = np.asarray(sin, dtype=np.float32)[:T_]
    w_qkv = np.asarray(w_qkv, dtype=np.float32)
    w_proj = np.asarray(w_proj, dtype=np.float32)
    g_q = np.asarray(g_q, dtype=np.float32)
    g_k = np.asarray(g_k, dtype=np.float32)

    srcidx = np.concatenate([np.arange(HD, D), np.arange(0, HD)])
    sign = np.concatenate([-np.ones(HD, np.float32), np.ones(HD, np.float32)])
    scale_q = 1.0 / math.sqrt(D)
    # transposed (d, t) rope tables; rotation sign folded into sin
    cqT_np = np.ascontiguousarray((cos * g_q[None, :] * scale_q).T).astype(NP_BF16)
    sqT_np = np.ascontiguousarray(
        (sin * sign[None, :] * g_q[srcidx][None, :] * scale_q).T
    ).astype(NP_BF16)
    ckT_np = np.ascontiguousarray((cos * g_k[None, :]).T).astype(NP_BF16)
    skT_np = np.ascontiguousarray((sin * sign[None, :] * g_k[srcidx][None, :]).T).astype(
        NP_BF16
    )

    wq_r = w_qkv.reshape(3, H, D, C)
    wp_r = w_proj.reshape(C, H, D)

    in_maps = []
    for core in range(NCORES):
        b = core // 2
        hg = core % 2
        xt_np = np.ascontiguousarray(x[b % b_count][:T_].T).astype(NP_BF16)
        wsel = wq_r[:, hg * HL : (hg + 1) * HL]  # [3, HL, D, C]
        # q,k as stationary lhsT tiles: [2, HL, C, D]
        wqk_np = np.ascontiguousarray(wsel[0:2].transpose(0, 1, 3, 2)).astype(NP_BF16)
        # v as moving rhs: [C, HL*D]
        wv_np = np.ascontiguousarray(
            wsel[2].transpose(2, 0, 1).reshape(C, HL * D)
        ).astype(NP_BF16)
        wp_np = np.ascontiguousarray(
            wp_r[:, hg * HL : (hg + 1) * HL, :].transpose(1, 2, 0)
        ).astype(NP_BF16)  # [HL, D, C]
        in_maps.append(
            {
                "xt": xt_np,
                "wqk": wqk_np,
                "wv": wv_np,
                "wp": wp_np,
                "cqT": cqT_np,
                "sqT": sqT_np,
                "ckT": ckT_np,
                "skT": skT_np,
            }
        )
    return in_maps


_nc_cache = {}


def _get_nc(T_=T):
    if T_ not in _nc_cache:
        _nc_cache[T_] = build(T_)
    return _nc_cache[T_]


def _install_trace_hook():
    """Register the axon NTFF profile hook (missing from this image's antenv)."""
    if "antenv.axon_hooks" in sys.modules:
        return
    try:
        from trn_agent_boot.trn_boot import _ntff_profile_via_ctypes

        hook = _ntff_profile_via_ctypes("/opt/axon/libaxon_pjrt.so")
        mod = types.ModuleType("antenv.axon_hooks")
        mod.get_axon_ntff_profile_hook = lambda: hook
        sys.modules["antenv.axon_hooks"] = mod
    except Exception:
        pass


def run(inputs, T_=T, trace=False, tmpdir=None):
    """Run the sharded kernel; returns (full output [B, T, C] fp32, results obj)."""
    nc = _get_nc(T_)
    in_maps = prep_inputs(**inputs, T_=T_)
    kwargs = {}
    if trace:
        _install_trace_hook()
        kwargs = dict(trace=True, tmpdir=tmpdir)
    res = run_bass_kernel_spmd(nc, in_maps, core_ids=list(range(NCORES)), **kwargs)
    outs = [np.asarray(res.results[i]["out"], dtype=np.float32) for i in range(NCORES)]
    full = np.stack([outs[2 * b] + outs[2 * b + 1] for b in range(B)], axis=0)
    return full, res


def kernel(x, cos, sin, w_qkv, w_proj, g_q, g_k):
    full, _ = run(
        dict(x=x, cos=cos, sin=sin, w_qkv=w_qkv, w_proj=w_proj, g_q=g_q, g_k=g_k)
    )
    return full
